# Initial kernel scaffold
#
"""MoE (top-2 of 8 experts) Trainium2 kernel — 8 NeuronCores.

Strategy (per sharding hint: expert parallelism + dispatch-by-routed-expert):
  Launch A (data-parallel gate): each core computes fp32 router
    logits/softmax/top-2 for its 1024-token shard on device, emitting the
    probs output plus per-token (top1, top2, w1, w2) routing records.
  Host dispatch: the device-computed routing records are reshaped into
    per-expert token index lists (pure index logistics — the all-to-all
    "sharding" step), padded to a static capacity.
  Launch B (expert-parallel MLP): core e owns expert e. It dma_gather's
    its routed tokens' rows from a replicated bf16 copy of x (transposed
    on the fly into [D, tokens] matmul layout), runs
    gelu(x@w1+b1)@w2+b2 in bf16 with fp32 accumulation, scales rows by the
    renormalized gate weight, and writes the dense per-expert result.
  Host combine: out[token] += y_expert[slot] (each (token, rank) pair is
    owned by exactly one expert core), probs shards are concatenated.
"""

import numpy as np
import ml_dtypes

import concourse.bacc as bacc
import concourse.mybir as mybir
import concourse.tile as tile
from concourse import bass_utils

F32 = mybir.dt.float32
BF16 = mybir.dt.bfloat16
I16 = mybir.dt.int16
AF = mybir.ActivationFunctionType
ALU = mybir.AluOpType
AX = mybir.AxisListType

E, D, H, K = 8, 512, 1024, 2
B, S = 4, 2048
N = B * S          # 8192 tokens
NC = 8             # cores
NSH = N // NC      # tokens per core in the gate launch
GT = NSH // 128    # token tiles per core in the gate launch (8)

_CACHE = {}


def _build_gate_nc():
    """Launch A: fp32 gate + softmax + top-2 for a 1024-token shard."""
    nc = bacc.Bacc("TRN2", target_bir_lowering=False, debug=False, num_devices=NC)
    xt = nc.dram_tensor("xt", [D, NSH], F32, kind="ExternalInput")
    gw = nc.dram_tensor("gw", [D, E], F32, kind="ExternalInput")
    gb = nc.dram_tensor("gb", [1, E], F32, kind="ExternalInput")
    iota = nc.dram_tensor("iota", [128, E], F32, kind="ExternalInput")
    ones = nc.dram_tensor("ones", [1, 128], F32, kind="ExternalInput")
    probs = nc.dram_tensor("probs", [NSH, E], F32, kind="ExternalOutput")
    route = nc.dram_tensor("route", [NSH, 4], F32, kind="ExternalOutput")

    KC = D // 128  # 4 contraction chunks

    with tile.TileContext(nc) as tc:
        with (
            tc.tile_pool(name="const", bufs=1) as cpool,
            tc.tile_pool(name="work", bufs=1) as wpool,
            tc.tile_pool(name="psum", bufs=1, space="PSUM") as ppool,
        ):
            xt_sb = cpool.tile([128, KC, NSH], F32)
            nc.sync.dma_start(xt_sb[:], xt.ap().rearrange("(c p) t -> p c t", p=128))
            gw_sb = cpool.tile([128, KC, E], F32)
            nc.sync.dma_start(gw_sb[:], gw.ap().rearrange("(c p) e -> p c e", p=128))
            gb_sb = cpool.tile([1, E], F32)
            nc.sync.dma_start(gb_sb[:], gb.ap())
            iota_sb = cpool.tile([128, E], F32)
            nc.sync.dma_start(iota_sb[:], iota.ap())
            ones_sb = cpool.tile([1, 128], F32)
            nc.sync.dma_start(ones_sb[:], ones.ap())

            lg = ppool.tile([128, GT, E], F32)  # all logits, one bank
            for ti in range(GT):
                for kc in range(KC):
                    nc.tensor.matmul(
                        lg[:, ti, :],
                        xt_sb[:, kc, ti * 128:(ti + 1) * 128],
                        gw_sb[:, kc, :],
                        start=(kc == 0),
                        stop=False,
                    )
                nc.tensor.matmul(
                    lg[:, ti, :], ones_sb[:, :], gb_sb[:, :], start=False, stop=True
                )

            ex = wpool.tile([128, GT, E], F32)
            nc.scalar.activation(ex[:], lg[:], AF.Exp)
            ssum = wpool.tile([128, GT], F32)
            nc.vector.tensor_reduce(ssum[:], ex[:], AX.X, ALU.add)
            rsum = wpool.tile([128, GT], F32)
            nc.vector.reciprocal(rsum[:], ssum[:])
            pr = wpool.tile([128, GT, E], F32)
            nc.vector.tensor_tensor(
                pr[:], ex[:], rsum[:, :, None].broadcast_to([128, GT, E]), ALU.mult
            )
            nc.sync.dma_start(probs.ap().rearrange("(g p) e -> p g e", p=128), pr[:])

            m1 = wpool.tile([128, GT], F32)
            nc.vector.tensor_reduce(m1[:], pr[:], AX.X, ALU.max)
            mask1 = wpool.tile([128, GT, E], F32)
            nc.vector.tensor_tensor(
                mask1[:], pr[:], m1[:, :, None].broadcast_to([128, GT, E]), ALU.is_ge
            )
            # notm = 1 - mask1 ; masked = pr * notm kills the argmax slot
            notm = wpool.tile([128, GT, E], F32)
            nc.vector.tensor_scalar(notm[:], mask1[:], -1.0, 1.0, ALU.mult, ALU.add)
            masked = wpool.tile([128, GT, E], F32)
            nc.vector.tensor_tensor(masked[:], pr[:], notm[:], ALU.mult)
            m2 = wpool.tile([128, GT], F32)
            nc.vector.tensor_reduce(m2[:], masked[:], AX.X, ALU.max)
            mask2 = wpool.tile([128, GT, E], F32)
            nc.vector.tensor_tensor(
                mask2[:], masked[:], m2[:, :, None].broadcast_to([128, GT, E]),
                ALU.is_ge,
            )
            # arg indices via max(mask * iota)
            t1 = wpool.tile([128, GT, E], F32)
            nc.vector.tensor_tensor(
                t1[:], mask1[:], iota_sb[:, None, :].broadcast_to([128, GT, E]),
                ALU.mult,
            )
            t2 = wpool.tile([128, GT, E], F32)
            nc.vector.tensor_tensor(
                t2[:], mask2[:], iota_sb[:, None, :].broadcast_to([128, GT, E]),
                ALU.mult,
            )
            rt = wpool.tile([128, GT, 4], F32)
            nc.vector.tensor_reduce(rt[:, :, 0], t1[:], AX.X, ALU.max)
            nc.vector.tensor_reduce(rt[:, :, 1], t2[:], AX.X, ALU.max)
            # renormalized top-2 weights
            wsum = wpool.tile([128, GT], F32)
            nc.vector.tensor_tensor(wsum[:], m1[:], m2[:], ALU.add)
            rw = wpool.tile([128, GT], F32)
            nc.vector.reciprocal(rw[:], wsum[:])
            nc.vector.tensor_tensor(rt[:, :, 2], m1[:], rw[:], ALU.mult)
            nc.vector.tensor_tensor(rt[:, :, 3], m2[:], rw[:], ALU.mult)
            nc.sync.dma_start(route.ap().rearrange("(g p) f -> p g f", p=128), rt[:])

    nc.compile()
    return nc


def _build_expert_nc(cap):
    """Launch B: one expert's MLP over `cap` gathered token slots."""
    nc = bacc.Bacc("TRN2", target_bir_lowering=False, debug=False, num_devices=NC)
    xb = nc.dram_tensor("xb", [N, D], BF16, kind="ExternalInput")
    idx = nc.dram_tensor("idx", [128, cap // 16], I16, kind="ExternalInput")
    gat = nc.dram_tensor("gat", [128, cap // 128], F32, kind="ExternalInput")
    w1 = nc.dram_tensor("w1", [D, H], BF16, kind="ExternalInput")
    w2 = nc.dram_tensor("w2", [H, D], BF16, kind="ExternalInput")
    b1 = nc.dram_tensor("b1", [128, H // 128], F32, kind="ExternalInput")
    b2 = nc.dram_tensor("b2", [1, D], F32, kind="ExternalInput")
    ones = nc.dram_tensor("ones", [1, 128], F32, kind="ExternalInput")
    y = nc.dram_tensor("y", [cap, D], F32, kind="ExternalOutput")

    KC = D // 128    # 4 k-chunks for layer 1
    MH = H // 128    # 8 h-tiles
    # layer-1 output chunks along the token axis (PSUM bank = 512 fp32)
    cns = [(s, min(512, cap - s)) for s in range(0, cap, 512)]
    TT = cap // 128  # token tiles for layer 2

    with tile.TileContext(nc) as tc:
        with (
            tc.tile_pool(name="const", bufs=1) as cpool,
            tc.tile_pool(name="xg", bufs=1) as xgpool,
            tc.tile_pool(name="h", bufs=1) as hpool,
            tc.tile_pool(name="y", bufs=3) as ypool,
            tc.tile_pool(name="ps1", bufs=4, space="PSUM") as ps1,
            tc.tile_pool(name="ps2", bufs=3, space="PSUM") as ps2,
        ):
            idx_sb = cpool.tile([128, cap // 16], I16)
            nc.sync.dma_start(idx_sb[:], idx.ap())
            gat_sb = cpool.tile([128, cap // 128], F32)
            nc.sync.dma_start(gat_sb[:], gat.ap())
            w1_sb = cpool.tile([128, KC, H], BF16)
            nc.sync.dma_start(w1_sb[:], w1.ap().rearrange("(c p) h -> p c h", p=128))
            w2_sb = cpool.tile([128, MH, D], BF16)
            nc.sync.dma_start(w2_sb[:], w2.ap().rearrange("(c p) d -> p c d", p=128))
            b1_sb = cpool.tile([128, MH], F32)
            nc.sync.dma_start(b1_sb[:], b1.ap())
            b2_sb = cpool.tile([1, D], F32)
            nc.sync.dma_start(b2_sb[:], b2.ap())
            ones_sb = cpool.tile([1, 128], F32)
            nc.sync.dma_start(ones_sb[:], ones.ap())

            # Gather x rows for this expert's tokens, transposed on the fly:
            # xg[p, c, s] = x[idx[s], c*128 + p]
            xg = xgpool.tile([128, KC, cap], BF16)
            nc.gpsimd.dma_gather(xg[:], xb.ap(), idx_sb[:], cap, cap, D)

            # Layer 1: hT[m*128+p, s] = gelu(sum_d x[s,d] w1[d, m*128+p] + b1)
            h_sb = hpool.tile([128, MH, cap], BF16)
            for m in range(MH):
                for cs, cw in cns:
                    ph = ps1.tile([128, 512], F32, tag="ph")
                    for kc in range(KC):
                        nc.tensor.matmul(
                            ph[:, :cw],
                            w1_sb[:, kc, m * 128:(m + 1) * 128],
                            xg[:, kc, cs:cs + cw],
                            start=(kc == 0),
                            stop=(kc == KC - 1),
                        )
                    nc.scalar.activation(
                        h_sb[:, m, cs:cs + cw], ph[:, :cw], AF.Gelu,
                        bias=b1_sb[:, m:m + 1],
                    )

            # Layer 2: y[t*128+p, :] = (hT[:, t*128+p] @ w2 + b2) * gating
            for t in range(TT):
                py = ps2.tile([128, D], F32, tag="py")
                nc.tensor.matmul(py[:], ones_sb[:], b2_sb[:], start=True, stop=False)
                for hc in range(MH):
                    nc.tensor.matmul(
                        py[:],
                        h_sb[:, hc, t * 128:(t + 1) * 128],
                        w2_sb[:, hc, :],
                        start=False,
                        stop=(hc == MH - 1),
                    )
                y_t = ypool.tile([128, D], F32, tag="yt")
                nc.vector.tensor_scalar_mul(y_t[:], py[:], gat_sb[:, t:t + 1])
                nc.sync.dma_start(y.ap()[t * 128:(t + 1) * 128, :], y_t[:])

    nc.compile()
    return nc


def _gate_nc():
    if "gate" not in _CACHE:
        _CACHE["gate"] = _build_gate_nc()
    return _CACHE["gate"]


def _expert_nc(cap):
    key = ("expert", cap)
    if key not in _CACHE:
        _CACHE[key] = _build_expert_nc(cap)
    return _CACHE[key]


def _run(nc, in_maps, **kw):
    return bass_utils.run_bass_kernel_spmd(
        nc, in_maps, core_ids=list(range(NC)), **kw
    )


def kernel(x, gate_w, gate_b, w1, b1, w2, b2, _timing=None):
    x = np.ascontiguousarray(np.asarray(x, np.float32))
    gate_w = np.ascontiguousarray(np.asarray(gate_w, np.float32))
    gate_b = np.ascontiguousarray(np.asarray(gate_b, np.float32)).reshape(1, E)
    w1 = np.asarray(w1, np.float32)
    b1 = np.asarray(b1, np.float32)
    w2 = np.asarray(w2, np.float32)
    b2 = np.asarray(b2, np.float32)

    xf = x.reshape(N, D)
    xT = np.ascontiguousarray(xf.T)                       # [D, N]
    ones = np.ones((1, 128), np.float32)
    iota = np.tile(np.arange(E, dtype=np.float32), (128, 1))

    # ---- Launch A: gate ----
    gnc = _gate_nc()
    in_maps = [
        {
            "xt": np.ascontiguousarray(xT[:, c * NSH:(c + 1) * NSH]),
            "gw": gate_w, "gb": gate_b, "iota": iota, "ones": ones,
        }
        for c in range(NC)
    ]
    resA = _run(gnc, in_maps)
    probs = np.concatenate([r["probs"] for r in resA.results], 0)  # [N, E]
    route = np.concatenate([r["route"] for r in resA.results], 0)  # [N, 4]

    # ---- Host dispatch: device-computed routing -> per-expert index lists ----
    pair_tok = np.tile(np.arange(N, dtype=np.int64), 2)
    pair_exp = np.concatenate([route[:, 0], route[:, 1]]).astype(np.int64)
    pair_w = np.concatenate([route[:, 2], route[:, 3]]).astype(np.float32)
    toks, ws, cnts = [], [], []
    for e in range(E):
        sel = pair_exp == e
        toks.append(pair_tok[sel])
        ws.append(pair_w[sel])
        cnts.append(int(sel.sum()))
    cap = max(512, -(-max(cnts) // 128) * 128)

    xb = np.ascontiguousarray(xf.astype(ml_dtypes.bfloat16))
    w1b = [np.ascontiguousarray(w1[e].astype(ml_dtypes.bfloat16)) for e in range(E)]
    w2b = [np.ascontiguousarray(w2[e].astype(ml_dtypes.bfloat16)) for e in range(E)]

    # ---- Launch B: expert MLPs ----
    enc = _expert_nc(cap)
    in_maps = []
    for e in range(E):
        idx_arr = np.zeros(cap, np.int16)
        idx_arr[:cnts[e]] = toks[e].astype(np.int16)
        gat_arr = np.zeros(cap, np.float32)
        gat_arr[:cnts[e]] = ws[e]
        in_maps.append({
            "xb": xb,
            "idx": np.ascontiguousarray(np.tile(idx_arr.reshape(-1, 16).T, (8, 1))),
            "gat": np.ascontiguousarray(gat_arr.reshape(-1, 128).T),
            "w1": w1b[e],
            "w2": w2b[e],
            "b1": np.ascontiguousarray(b1[e].reshape(-1, 128).T),
            "b2": np.ascontiguousarray(b2[e].reshape(1, D)),
            "ones": ones,
        })
    resB = _run(enc, in_maps)

    # ---- Host combine (inverse of the dispatch permutation) ----
    out = np.zeros((N, D), np.float32)
    for e in range(E):
        ye = resB.results[e]["y"]
        out[toks[e]] += ye[:cnts[e]]

    if _timing is not None:
        _timing["gate_nc"] = gnc
        _timing["expert_nc"] = enc
        _timing["gate_in_maps"] = in_maps if False else None
    return out.reshape(B, S, D), probs.reshape(B, S, E)


# revision 9
# speedup vs baseline: 1.1129x; 1.1129x over previous
"""MoE (top-2 of 8 experts) Trainium2 kernel — 8 NeuronCores.

Strategy (per sharding hint: expert parallelism + dispatch-by-routed-expert):
  Launch A (data-parallel gate): each core computes fp32 router
    logits/softmax/top-2 for its 1024-token shard on device, emitting the
    probs output plus per-token (top1, top2, w1, w2) routing records.
  Host dispatch: the device-computed routing records are reshaped into
    per-expert token index lists (pure index logistics — the all-to-all
    "sharding" step), padded to a static capacity.
  Launch B (expert-parallel MLP): core e owns expert e. It dma_gather's
    its routed tokens' rows from a replicated bf16 copy of x (transposed
    on the fly into [D, tokens] matmul layout), runs
    gelu(x@w1+b1)@w2+b2 in bf16 with fp32 accumulation, scales rows by the
    renormalized gate weight, and writes the dense per-expert result.
  Host combine: out[token] += y_expert[slot] (each (token, rank) pair is
    owned by exactly one expert core), probs shards are concatenated.
"""

import numpy as np
import ml_dtypes

import concourse.bacc as bacc
import concourse.mybir as mybir
import concourse.tile as tile
from concourse import bass_utils

F32 = mybir.dt.float32
BF16 = mybir.dt.bfloat16
I16 = mybir.dt.int16
AF = mybir.ActivationFunctionType
ALU = mybir.AluOpType
AX = mybir.AxisListType

E, D, H, K = 8, 512, 1024, 2
B, S = 4, 2048
N = B * S          # 8192 tokens
NC = 8             # cores
NSH = N // NC      # tokens per core in the gate launch
GT = NSH // 128    # token tiles per core in the gate launch (8)

_CACHE = {}


def _build_gate_nc():
    """Launch A: fp32 gate + softmax + top-2 for a 1024-token shard."""
    nc = bacc.Bacc("TRN2", target_bir_lowering=False, debug=False, num_devices=NC)
    xt = nc.dram_tensor("xt", [D, NSH], F32, kind="ExternalInput")
    gw = nc.dram_tensor("gw", [D, E], F32, kind="ExternalInput")
    gb = nc.dram_tensor("gb", [1, E], F32, kind="ExternalInput")
    iota = nc.dram_tensor("iota", [128, E], F32, kind="ExternalInput")
    ones = nc.dram_tensor("ones", [1, 128], F32, kind="ExternalInput")
    probs = nc.dram_tensor("probs", [NSH, E], F32, kind="ExternalOutput")
    route = nc.dram_tensor("route", [NSH, 4], F32, kind="ExternalOutput")

    KC = D // 128  # 4 contraction chunks

    with tile.TileContext(nc) as tc:
        with (
            tc.tile_pool(name="const", bufs=1) as cpool,
            tc.tile_pool(name="work", bufs=1) as wpool,
            tc.tile_pool(name="psum", bufs=1, space="PSUM") as ppool,
        ):
            xt_sb = cpool.tile([128, KC, NSH], F32)
            nc.sync.dma_start(xt_sb[:], xt.ap().rearrange("(c p) t -> p c t", p=128))
            gw_sb = cpool.tile([128, KC, E], F32)
            nc.sync.dma_start(gw_sb[:], gw.ap().rearrange("(c p) e -> p c e", p=128))
            gb_sb = cpool.tile([1, E], F32)
            nc.sync.dma_start(gb_sb[:], gb.ap())
            iota_sb = cpool.tile([128, E], F32)
            nc.sync.dma_start(iota_sb[:], iota.ap())
            ones_sb = cpool.tile([1, 128], F32)
            nc.sync.dma_start(ones_sb[:], ones.ap())

            lg = ppool.tile([128, GT, E], F32)  # all logits, one bank
            for ti in range(GT):
                for kc in range(KC):
                    nc.tensor.matmul(
                        lg[:, ti, :],
                        xt_sb[:, kc, ti * 128:(ti + 1) * 128],
                        gw_sb[:, kc, :],
                        start=(kc == 0),
                        stop=False,
                    )
                nc.tensor.matmul(
                    lg[:, ti, :], ones_sb[:, :], gb_sb[:, :], start=False, stop=True
                )

            ex = wpool.tile([128, GT, E], F32)
            nc.scalar.activation(ex[:], lg[:], AF.Exp)
            ssum = wpool.tile([128, GT], F32)
            nc.vector.tensor_reduce(ssum[:], ex[:], AX.X, ALU.add)
            rsum = wpool.tile([128, GT], F32)
            nc.vector.reciprocal(rsum[:], ssum[:])
            pr = wpool.tile([128, GT, E], F32)
            nc.vector.tensor_tensor(
                pr[:], ex[:], rsum[:, :, None].broadcast_to([128, GT, E]), ALU.mult
            )
            nc.sync.dma_start(probs.ap().rearrange("(g p) e -> p g e", p=128), pr[:])

            m1 = wpool.tile([128, GT], F32)
            nc.vector.tensor_reduce(m1[:], pr[:], AX.X, ALU.max)
            mask1 = wpool.tile([128, GT, E], F32)
            nc.vector.tensor_tensor(
                mask1[:], pr[:], m1[:, :, None].broadcast_to([128, GT, E]), ALU.is_ge
            )
            # notm = 1 - mask1 ; masked = pr * notm kills the argmax slot
            notm = wpool.tile([128, GT, E], F32)
            nc.vector.tensor_scalar(notm[:], mask1[:], -1.0, 1.0, ALU.mult, ALU.add)
            masked = wpool.tile([128, GT, E], F32)
            nc.vector.tensor_tensor(masked[:], pr[:], notm[:], ALU.mult)
            m2 = wpool.tile([128, GT], F32)
            nc.vector.tensor_reduce(m2[:], masked[:], AX.X, ALU.max)
            mask2 = wpool.tile([128, GT, E], F32)
            nc.vector.tensor_tensor(
                mask2[:], masked[:], m2[:, :, None].broadcast_to([128, GT, E]),
                ALU.is_ge,
            )
            # arg indices via max(mask * iota)
            t1 = wpool.tile([128, GT, E], F32)
            nc.vector.tensor_tensor(
                t1[:], mask1[:], iota_sb[:, None, :].broadcast_to([128, GT, E]),
                ALU.mult,
            )
            t2 = wpool.tile([128, GT, E], F32)
            nc.vector.tensor_tensor(
                t2[:], mask2[:], iota_sb[:, None, :].broadcast_to([128, GT, E]),
                ALU.mult,
            )
            rt = wpool.tile([128, GT, 4], F32)
            nc.vector.tensor_reduce(rt[:, :, 0], t1[:], AX.X, ALU.max)
            nc.vector.tensor_reduce(rt[:, :, 1], t2[:], AX.X, ALU.max)
            # renormalized top-2 weights
            wsum = wpool.tile([128, GT], F32)
            nc.vector.tensor_tensor(wsum[:], m1[:], m2[:], ALU.add)
            rw = wpool.tile([128, GT], F32)
            nc.vector.reciprocal(rw[:], wsum[:])
            nc.vector.tensor_tensor(rt[:, :, 2], m1[:], rw[:], ALU.mult)
            nc.vector.tensor_tensor(rt[:, :, 3], m2[:], rw[:], ALU.mult)
            nc.sync.dma_start(route.ap().rearrange("(g p) f -> p g f", p=128), rt[:])

    nc.compile()
    return nc


def _build_expert_nc(cap, act=AF.Gelu, reps=1):
    """Launch B: one expert's MLP over `cap` gathered token slots.

    reps>1 re-executes the body in a hardware loop (benchmark mode)."""
    nc = bacc.Bacc("TRN2", target_bir_lowering=False, debug=False, num_devices=NC)
    xb = nc.dram_tensor("xb", [N, D], BF16, kind="ExternalInput")
    idx = nc.dram_tensor("idx", [128, cap // 16], I16, kind="ExternalInput")
    gat = nc.dram_tensor("gat", [128, cap // 128], F32, kind="ExternalInput")
    w1 = nc.dram_tensor("w1", [D, H], BF16, kind="ExternalInput")
    w2 = nc.dram_tensor("w2", [H, D], BF16, kind="ExternalInput")
    b1 = nc.dram_tensor("b1", [128, H // 128], F32, kind="ExternalInput")
    b2 = nc.dram_tensor("b2", [1, D], F32, kind="ExternalInput")
    ones = nc.dram_tensor("ones", [1, 128], F32, kind="ExternalInput")
    y = nc.dram_tensor("y", [cap, D], F32, kind="ExternalOutput")

    KC = D // 128    # 4 k-chunks for layer 1
    MH = H // 128    # 8 h-tiles
    # layer-1 output chunks along the token axis (PSUM bank = 512 fp32)
    cns = [(s, min(512, cap - s)) for s in range(0, cap, 512)]
    TT = cap // 128  # token tiles for layer 2

    with tile.TileContext(nc) as tc:
        with (
            tc.tile_pool(name="const", bufs=1) as cpool,
            tc.tile_pool(name="xg", bufs=1) as xgpool,
            tc.tile_pool(name="h", bufs=1) as hpool,
            tc.tile_pool(name="y", bufs=3) as ypool,
            tc.tile_pool(name="ps1", bufs=4, space="PSUM") as ps1,
            tc.tile_pool(name="ps2", bufs=3, space="PSUM") as ps2,
        ):
            idx_sb = cpool.tile([128, cap // 16], I16)
            nc.sync.dma_start(idx_sb[:], idx.ap())
            gat_sb = cpool.tile([128, cap // 128], F32)
            nc.sync.dma_start(gat_sb[:], gat.ap())
            w1_sb = cpool.tile([128, KC, H], BF16)
            nc.sync.dma_start(w1_sb[:], w1.ap().rearrange("(c p) h -> p c h", p=128))
            w2_sb = cpool.tile([128, MH, D], BF16)
            nc.sync.dma_start(w2_sb[:], w2.ap().rearrange("(c p) d -> p c d", p=128))
            b1_sb = cpool.tile([128, MH], F32)
            nc.sync.dma_start(b1_sb[:], b1.ap())
            b2_sb = cpool.tile([1, D], F32)
            nc.sync.dma_start(b2_sb[:], b2.ap())
            ones_sb = cpool.tile([1, 128], F32)
            nc.sync.dma_start(ones_sb[:], ones.ap())

            def body():
                # Gather x rows for this expert's tokens, transposed on the
                # fly: xg[p, c, s] = x[idx[s], c*128 + p]
                xg = xgpool.tile([128, KC, cap], BF16, tag="xg")
                nc.gpsimd.dma_gather(
                    xg[:], xb.ap(), idx_sb[:], cap, cap, D, transpose=True,
                    single_packet=False,
                )

                # Layer 1: hT[m*128+p, s] = gelu(sum_d x[s,d] w1[d,m*128+p]+b1)
                h_sb = hpool.tile([128, MH, cap], BF16, tag="h")
                for m in range(MH):
                    for cs, cw in cns:
                        ph = ps1.tile([128, 512], F32, tag="ph")
                        for kc in range(KC):
                            nc.tensor.matmul(
                                ph[:, :cw],
                                w1_sb[:, kc, m * 128:(m + 1) * 128],
                                xg[:, kc, cs:cs + cw],
                                start=(kc == 0),
                                stop=(kc == KC - 1),
                            )
                        nc.scalar.activation(
                            h_sb[:, m, cs:cs + cw], ph[:, :cw], act,
                            bias=b1_sb[:, m:m + 1],
                        )

                # Layer 2: y[t*128+p, :] = (hT[:, t*128+p] @ w2 + b2) * gating
                for t in range(TT):
                    py = ps2.tile([128, D], F32, tag="py")
                    nc.tensor.matmul(
                        py[:], ones_sb[:], b2_sb[:], start=True, stop=False
                    )
                    for hc in range(MH):
                        nc.tensor.matmul(
                            py[:],
                            h_sb[:, hc, t * 128:(t + 1) * 128],
                            w2_sb[:, hc, :],
                            start=False,
                            stop=(hc == MH - 1),
                        )
                    y_t = ypool.tile([128, D], F32, tag="yt")
                    nc.vector.tensor_scalar_mul(y_t[:], py[:], gat_sb[:, t:t + 1])
                    nc.sync.dma_start(y.ap()[t * 128:(t + 1) * 128, :], y_t[:])

            if reps == 1:
                body()
            else:
                with tc.For_i(0, reps, 1):
                    body()

    nc.compile()
    return nc


def _gate_nc():
    if "gate" not in _CACHE:
        _CACHE["gate"] = _build_gate_nc()
    return _CACHE["gate"]


def _expert_nc(cap):
    key = ("expert", cap)
    if key not in _CACHE:
        _CACHE[key] = _build_expert_nc(cap)
    return _CACHE[key]


def _run(nc, in_maps, **kw):
    return bass_utils.run_bass_kernel_spmd(
        nc, in_maps, core_ids=list(range(NC)), **kw
    )


def kernel(x, gate_w, gate_b, w1, b1, w2, b2):
    x = np.ascontiguousarray(np.asarray(x, np.float32))
    gate_w = np.ascontiguousarray(np.asarray(gate_w, np.float32))
    gate_b = np.ascontiguousarray(np.asarray(gate_b, np.float32)).reshape(1, E)
    w1 = np.asarray(w1, np.float32)
    b1 = np.asarray(b1, np.float32)
    w2 = np.asarray(w2, np.float32)
    b2 = np.asarray(b2, np.float32)

    xf = x.reshape(N, D)
    xT = np.ascontiguousarray(xf.T)                       # [D, N]
    ones = np.ones((1, 128), np.float32)
    iota = np.tile(np.arange(E, dtype=np.float32), (128, 1))

    # ---- Launch A: gate ----
    gnc = _gate_nc()
    in_maps = [
        {
            "xt": np.ascontiguousarray(xT[:, c * NSH:(c + 1) * NSH]),
            "gw": gate_w, "gb": gate_b, "iota": iota, "ones": ones,
        }
        for c in range(NC)
    ]
    resA = _run(gnc, in_maps)
    probs = np.concatenate([r["probs"] for r in resA.results], 0)  # [N, E]
    route = np.concatenate([r["route"] for r in resA.results], 0)  # [N, 4]

    # ---- Host dispatch: device-computed routing -> per-expert index lists ----
    pair_tok = np.tile(np.arange(N, dtype=np.int64), 2)
    pair_exp = np.concatenate([route[:, 0], route[:, 1]]).astype(np.int64)
    pair_w = np.concatenate([route[:, 2], route[:, 3]]).astype(np.float32)
    toks, ws, cnts = [], [], []
    for e in range(E):
        sel = pair_exp == e
        toks.append(pair_tok[sel])
        ws.append(pair_w[sel])
        cnts.append(int(sel.sum()))
    cap = max(512, -(-max(cnts) // 128) * 128)

    xb = np.ascontiguousarray(xf.astype(ml_dtypes.bfloat16))
    w1b = [np.ascontiguousarray(w1[e].astype(ml_dtypes.bfloat16)) for e in range(E)]
    w2b = [np.ascontiguousarray(w2[e].astype(ml_dtypes.bfloat16)) for e in range(E)]

    # ---- Launch B: expert MLPs ----
    enc = _expert_nc(cap)
    in_maps = []
    for e in range(E):
        idx_arr = np.zeros(cap, np.int16)
        idx_arr[:cnts[e]] = toks[e].astype(np.int16)
        gat_arr = np.zeros(cap, np.float32)
        gat_arr[:cnts[e]] = ws[e]
        in_maps.append({
            "xb": xb,
            "idx": np.ascontiguousarray(np.tile(idx_arr.reshape(-1, 16).T, (8, 1))),
            "gat": np.ascontiguousarray(gat_arr.reshape(-1, 128).T),
            "w1": w1b[e],
            "w2": w2b[e],
            "b1": np.ascontiguousarray(b1[e].reshape(-1, 128).T),
            "b2": np.ascontiguousarray(b2[e].reshape(1, D)),
            "ones": ones,
        })
    resB = _run(enc, in_maps)

    # ---- Host combine (inverse of the dispatch permutation) ----
    out = np.zeros((N, D), np.float32)
    for e in range(E):
        ye = resB.results[e]["y"]
        out[toks[e]] += ye[:cnts[e]]

    return out.reshape(B, S, D), probs.reshape(B, S, E)
